# revision 18
# baseline (speedup 1.0000x reference)
"""Trainium2 Bass kernel for single-level deformable attention.

Problem: nn_DeformableAttention (B=4, Q=S=10000, D=256, NH=8, NP=4, H=W=100).

Sharding: 8 cores = batch(4) x head-group(2).  Each core computes one batch
item with 4 heads (128 of the 256 value channels); the output projection is
row-parallel, so each core produces a full [Q, 256] partial output and the
host sums the two partials per batch item (+ b_out).

Per-core algorithm:
  1. Value projection V = E @ Wv_slice.T  -> [S, 128] (4 heads x 32ch),
     written to a DRAM "tall-pair" map per head: T[m] = [v[m-101], v[m-1]]
     (64 floats = 256B per row).  One 512B dma_gather descriptor starting at
     row a = 101 + y0*W + x0 then covers all four bilinear corners:
     T[a] = [v(y0,x0), v(y1,x0)],  T[a+1] = [v(y0,x1), v(y1,x1)].
  2. Offsets+attention projection (PE), bilinear weights / softmax /
     anchor indices (DVE/ACT), anchor fold+replicate to the dma_gather
     index layout (PE matmuls with permutation matrices).
  3. dma_gather (GPSIMD SWDGE) of 512B corner blocks, weighted combine
     (DVE/GPSIMD multiplies + strided reduces), PE transpose + output
     projection matmul, DMA out.
"""

import numpy as np

# ---------------------------------------------------------------- config

def make_cfg(H=100, W=100, Q=10000, U=4):
    S = H * W
    FAT = 128 * U                      # queries per fat tile
    NQT = -(-Q // FAT)                 # fat tiles
    QP = NQT * FAT                     # padded queries
    NSB = -(-S // 512)                 # 512-row superblocks for value proj
    SP = NSB * 512                     # padded spatial size
    TR = SP + 256                      # tall-map rows per head
    NG = 16                            # (4 heads/core) x (4 points)
    GRP = NG * U                       # gather groups per fat tile
    return dict(H=H, W=W, Q=Q, S=S, U=U, FAT=FAT, NQT=NQT, QP=QP,
                NSB=NSB, SP=SP, TR=TR, NG=NG, GRP=GRP,
                D=256, DC=128, NHC=4, NP=4, d=32)


CFG_FULL = make_cfg()

MAGIC = 12582912.0                     # 1.5 * 2**23, round-to-int trick


# ---------------------------------------------------------------- builder

def build(cfg):
    """Emit the per-core Bass program (SPMD, identical on all 8 cores)."""
    import concourse.bass as bass
    import concourse.bacc as bacc
    import concourse.mybir as mybir
    from concourse import tile

    f32 = mybir.dt.float32
    i16 = mybir.dt.int16
    Alu = mybir.AluOpType
    Act = mybir.ActivationFunctionType
    AX = mybir.AxisListType

    H, W = cfg["H"], cfg["W"]
    U, FAT, NQT = cfg["U"], cfg["FAT"], cfg["NQT"]
    NSB, SP, TR = cfg["NSB"], cfg["SP"], cfg["TR"]
    NG, GRP = cfg["NG"], cfg["GRP"]
    D, DC = cfg["D"], cfg["DC"]
    QP = cfg["QP"]
    NIH = 8 * U * 128                  # num_idxs per gather call (half heads)

    nc = bacc.Bacc()

    hid = nc.declare_dram_parameter("hidden", [QP, D], f32, isOutput=False)
    enc = nc.declare_dram_parameter("encoder", [SP, D], f32, isOutput=False)
    ref = nc.declare_dram_parameter("ref", [QP, 2], f32, isOutput=False)
    wofa = nc.declare_dram_parameter("wofa", [D, 48], f32, isOutput=False)
    bofa = nc.declare_dram_parameter("bofa", [1, 48], f32, isOutput=False)
    wv = nc.declare_dram_parameter("wv", [D, DC], f32, isOutput=False)
    wo = nc.declare_dram_parameter("wo", [DC, D], f32, isOutput=False)
    sels_in = nc.declare_dram_parameter("sels", [128, 128], f32, isOutput=False)
    rep_in = nc.declare_dram_parameter("rep", [16, 128], f32, isOutput=False)
    idn_in = nc.declare_dram_parameter("idn", [128, 128], f32, isOutput=False)
    hoff_in = nc.declare_dram_parameter("hoffb", [128, GRP], f32, isOutput=False)
    outp = nc.declare_dram_parameter("outp", [QP, D], f32, isOutput=True)
    DBG = bool(cfg.get("debug"))
    if DBG:
        dbg_v = nc.declare_dram_parameter("dbg_v", [SP, DC], f32, isOutput=True)
        dbg_oa = nc.declare_dram_parameter("dbg_oa", [128, U, 48], f32, isOutput=True)
        dbg_ang = nc.declare_dram_parameter("dbg_ang", [128, GRP], f32, isOutput=True)
        dbg_i16 = nc.declare_dram_parameter("dbg_i16", [128, GRP * 8], i16, isOutput=True)
        dbg_g = nc.declare_dram_parameter("dbg_g", [128, GRP, 128], f32, isOutput=True)
        dbg_w4 = nc.declare_dram_parameter("dbg_w4", [128, NG, U, 4], f32, isOutput=True)
        dbg_smp = nc.declare_dram_parameter("dbg_smp", [128, U, 4, 32], f32, isOutput=True)

    with tile.TileContext(nc) as tc:
        with (
            tc.tile_pool(name="consts", bufs=1) as cpool,
            tc.tile_pool(name="vwork", bufs=2) as vpool,
            tc.tile_pool(name="qwork", bufs=2) as qpool,
            tc.tile_pool(name="b3", bufs=2) as bpool,
            tc.tile_pool(name="gbuf", bufs=2) as gpool,
            tc.tile_pool(name="mbuf", bufs=2) as mpool,
            tc.tile_pool(name="ps_sm", bufs=3, space="PSUM") as ps_sm,
            tc.tile_pool(name="ps_wide", bufs=2, space="PSUM") as ps_wide,
            tc.tile_pool(name="ps_v", bufs=2, space="PSUM") as ps_v,
            tc.tile_pool(name="dram", bufs=1, space="DRAM") as dpool,
        ):
            # ---------------- constants
            wofa_sb = cpool.tile([128, 2, 48], f32, tag="c_wofa")
            nc.sync.dma_start(wofa_sb[:], wofa[:].rearrange("(k p) c -> p k c", p=128))
            bofa_sb = cpool.tile([1, 48], f32, tag="c_bofa")
            nc.sync.dma_start(bofa_sb[:], bofa[:])
            wv_sb = cpool.tile([128, 2, DC], f32, tag="c_wv")
            nc.sync.dma_start(wv_sb[:], wv[:].rearrange("(k p) c -> p k c", p=128))
            wo_sb = cpool.tile([DC, D], f32, tag="c_wo")
            nc.sync.dma_start(wo_sb[:], wo[:])
            sels = cpool.tile([128, 128], f32, tag="c_sels")
            nc.sync.dma_start(sels[:], sels_in[:])
            rep = cpool.tile([16, 128], f32, tag="c_rep")
            nc.sync.dma_start(rep[:], rep_in[:])
            idn = cpool.tile([128, 128], f32, tag="c_idn")
            nc.sync.dma_start(idn[:], idn_in[:])
            hoffb = cpool.tile([128, GRP], f32, tag="c_hoff")
            nc.sync.dma_start(hoffb[:], hoff_in[:])
            ones = cpool.tile([1, FAT], f32, tag="c_ones")
            nc.vector.memset(ones[:], 1.0)
            zeros = cpool.tile([128, 64], f32, tag="c_zeros")
            nc.vector.memset(zeros[:], 0.0)
            nc.const_aps.aps[(f32, 0.0)] = zeros[:, 0:1]

            # tall-pair value maps: one DRAM tile per head pair, [2*TR, 64]
            tmap = [dpool.tile([2 * TR, 64], f32, tag=f"tmap{p}", name=f"tmap{p}")
                    for p in range(2)]

            # zero-init the tall-map rows that can be gathered with zero
            # weight but are never written by the value projection
            for p in range(2):
                for wh in range(2):
                    base = wh * TR
                    for lo, hi in ((0, W + 1), (1 + SP, TR)):
                        r = lo
                        while r < hi:
                            n = min(128, hi - r)
                            nc.sync.dma_start(
                                tmap[p][:][base + r:base + r + n, :], zeros[:n, :])
                            r += n

            # ---------------- phase V: value projection -> tall maps
            enc_v = enc[:].rearrange("(sb st p) d -> sb p st d", st=4, p=128)
            for sb in range(NSB):
                e4 = vpool.tile([128, 4, D], f32, tag="e4")
                nc.sync.dma_start(e4[:], enc_v[sb])
                v4 = ps_v.tile([128, 4, DC], f32, tag="psv")
                for st in range(4):
                    for k in range(2):
                        pt_ = ps_sm.tile([128, 128], f32, tag="pssm")
                        nc.tensor.transpose(pt_[:], e4[:, st, k * 128:(k + 1) * 128], idn[:])
                        et = vpool.tile([128, 128], f32, tag="et")
                        nc.scalar.copy(et[:], pt_[:])
                        nc.tensor.matmul(v4[:, st, :], et[:], wv_sb[:, k, :],
                                         start=(k == 0), stop=(k == 1))
                v4s = vpool.tile([128, 4, DC], f32, tag="v4s")
                nc.vector.tensor_copy(v4s[:], v4[:])
                if DBG:
                    nc.sync.dma_start(
                        dbg_v[:].rearrange("(sb st p) c -> sb p st c", st=4, p=128)[sb],
                        v4s[:])
                # scatter to tall maps: per pair, half1 rows +101 cols 0:32,
                # half2 rows +1 cols 32:64; heads w=0,1 within pair
                for p in range(2):
                    for w in range(2):
                        src = v4s[:, :, (p * 2 + w) * 32:(p * 2 + w + 1) * 32]
                        for roff, coff in ((W + 1, 0), (1, 32)):
                            dst = bass.AP(
                                tmap[p][:].tensor,
                                (w * TR + roff + sb * 512) * 64 + coff,
                                [[64, 128], [128 * 64, 4], [1, 32]],
                            )
                            nc.sync.dma_start(dst, src)

            # ---------------- phase Q: per fat tile
            hid_v = hid[:].rearrange("(t u p) d -> t p u d", u=U, p=128)
            ref_v = ref[:].rearrange("(t u p) c -> t p u c", u=U, p=128)
            out_v = outp[:].rearrange("(t u p) d -> t p u d", u=U, p=128)

            for ft in range(NQT):
                hf = qpool.tile([128, U, D], f32, tag="hf")
                nc.sync.dma_start(hf[:], hid_v[ft])
                rf = qpool.tile([128, U, 2], f32, tag="rf")
                nc.sync.dma_start(rf[:], ref_v[ft])

                # B1: transpose hidden, project offsets+attention
                ht = [qpool.tile([128, U, 128], f32, tag=f"ht{k}", name=f"ht{k}")
                      for k in range(2)]
                for u in range(U):
                    for k in range(2):
                        pt_ = ps_sm.tile([128, 128], f32, tag="pssm")
                        nc.tensor.transpose(pt_[:], hf[:, u, k * 128:(k + 1) * 128], idn[:])
                        nc.scalar.copy(ht[k][:, u, :], pt_[:])
                poa = ps_wide.tile([48, FAT], f32, tag="pswide")
                nc.tensor.matmul(poa[:], wofa_sb[:, 0, :],
                                 ht[0][:].rearrange("p u c -> p (u c)"),
                                 start=True, stop=False)
                nc.tensor.matmul(poa[:], wofa_sb[:, 1, :],
                                 ht[1][:].rearrange("p u c -> p (u c)"),
                                 start=False, stop=False)
                nc.tensor.matmul(poa[:], bofa_sb[:], ones[:], start=False, stop=True)
                oat = qpool.tile([48, FAT], f32, tag="oat")
                nc.vector.tensor_copy(oat[:], poa[:])

                # B2: transpose back to query-partitioned OFF/ATT tiles
                off_t = qpool.tile([128, U, 16, 2], f32, tag="off_t")
                att = qpool.tile([128, U, 16], f32, tag="att")
                for u in range(U):
                    pt_ = ps_sm.tile([128, 48], f32, tag="pssm")
                    nc.tensor.transpose(pt_[:], oat[:, u * 128:(u + 1) * 128], idn[:48, :48])
                    nc.scalar.copy(off_t[:, u], pt_[:, 0:32].rearrange("p (g c) -> p g c", c=2))
                    nc.scalar.copy(att[:, u, :], pt_[:, 32:48])

                # B3: bilinear weights / softmax / anchors
                # rb = ref*100 - 0.5 : [128, U, 2]
                rb = bpool.tile([128, U, 2], f32, tag="rb")
                nc.vector.tensor_scalar(rb[:], rf[:], float(W), -0.5, Alu.mult, Alu.add)
                # XY = off + rb, x and y separately (3-dim APs)
                xy = bpool.tile([128, U, NG, 2], f32, tag="xy")
                for c in range(2):
                    nc.vector.tensor_tensor(
                        xy[:, :, :, c], off_t[:, :, :, c],
                        rb[:, :, c].unsqueeze(2).broadcast_to([128, U, NG]),
                        Alu.add)
                # XY0 = floor(xy) via round(x) - (round(x) > x)
                xyr = bpool.tile([128, U, NG, 2], f32, tag="xyr")
                nc.vector.tensor_scalar(xyr[:], xy[:], MAGIC, -MAGIC, Alu.add, Alu.add)
                gt = bpool.tile([128, U, NG, 2], f32, tag="gt")
                nc.vector.tensor_tensor(gt[:], xyr[:], xy[:], Alu.is_gt)
                xy0 = bpool.tile([128, U, NG, 2], f32, tag="xy0")
                nc.vector.tensor_tensor(xy0[:], xyr[:], gt[:], Alu.subtract)
                # interpolation weights
                w1 = bpool.tile([128, U, NG, 2], f32, tag="w1")
                nc.vector.tensor_tensor(w1[:], xy[:], xy0[:], Alu.subtract)
                w0 = bpool.tile([128, U, NG, 2], f32, tag="w0")
                nc.vector.tensor_scalar(w0[:], w1[:], -1.0, 1.0, Alu.mult, Alu.add)
                # validity of corner 0 and corner 1 (bounds identical for x/y)
                va = bpool.tile([128, U, NG, 2], f32, tag="va")
                nc.vector.tensor_scalar(va[:], xy0[:], 0.0, 0.0, Alu.is_ge, Alu.add)
                v0 = bpool.tile([128, U, NG, 2], f32, tag="v0")
                nc.vector.scalar_tensor_tensor(v0[:], xy0[:], float(W - 1), va[:],
                                               Alu.is_le, Alu.mult)
                nc.vector.tensor_scalar(va[:], xy0[:], -1.0, 0.0, Alu.is_ge, Alu.add)
                v1 = bpool.tile([128, U, NG, 2], f32, tag="v1")
                nc.vector.scalar_tensor_tensor(v1[:], xy0[:], float(W - 2), va[:],
                                               Alu.is_le, Alu.mult)
                u0 = bpool.tile([128, U, NG, 2], f32, tag="u0")
                nc.vector.tensor_tensor(u0[:], w0[:], v0[:], Alu.mult)
                u1 = bpool.tile([128, U, NG, 2], f32, tag="u1")
                nc.vector.tensor_tensor(u1[:], w1[:], v1[:], Alu.mult)
                # softmax over the 4 points of each head ((u,h) merged)
                lgv = att[:].rearrange("p u (h t) -> p (u h) t", t=4)
                mx = bpool.tile([128, U * 4], f32, tag="mx")
                nc.vector.tensor_reduce(mx[:], lgv, AX.X, Alu.max)
                le = bpool.tile([128, U * 4, 4], f32, tag="le")
                nc.vector.tensor_tensor(
                    le[:], lgv,
                    mx[:].unsqueeze(2).broadcast_to([128, U * 4, 4]),
                    Alu.subtract)
                ex = bpool.tile([128, U * 4, 4], f32, tag="ex")
                nc.scalar.activation(ex[:], le[:], Act.Exp)
                sm = bpool.tile([128, U * 4], f32, tag="sm")
                nc.vector.tensor_reduce(sm[:], ex[:], AX.X, Alu.add)
                rs = bpool.tile([128, U * 4], f32, tag="rs")
                nc.vector.reciprocal(rs[:], sm[:])
                at = bpool.tile([128, U * 4, 4], f32, tag="at")
                nc.vector.tensor_tensor(
                    at[:], ex[:],
                    rs[:].unsqueeze(2).broadcast_to([128, U * 4, 4]),
                    Alu.mult)
                atg = at[:].rearrange("p (u h) t -> p u (h t)", u=U)
                # fold attention into the y-interpolation weights
                ay0 = bpool.tile([128, U, NG], f32, tag="ay0")
                nc.vector.tensor_tensor(ay0[:], u0[:, :, :, 1], atg, Alu.mult)
                ay1 = bpool.tile([128, U, NG], f32, tag="ay1")
                nc.vector.tensor_tensor(ay1[:], u1[:, :, :, 1], atg, Alu.mult)
                # W4[p, grp, c] with grp = g*U + u, corners (y0x0,y1x0,y0x1,y1x1)
                w4 = bpool.tile([128, NG, U, 4], f32, tag="w4")
                w4v = w4[:].rearrange("p g u c -> p u g c")
                nc.vector.tensor_tensor(w4v[:, :, :, 0], ay0[:], u0[:, :, :, 0], Alu.mult)
                nc.vector.tensor_tensor(w4v[:, :, :, 1], ay1[:], u0[:, :, :, 0], Alu.mult)
                nc.vector.tensor_tensor(w4v[:, :, :, 2], ay0[:], u1[:, :, :, 0], Alu.mult)
                nc.vector.tensor_tensor(w4v[:, :, :, 3], ay1[:], u1[:, :, :, 0], Alu.mult)
                # anchors: clip coords, an = cy*W + cx + (W+1) + head offset
                cxy = bpool.tile([128, U, NG, 2], f32, tag="cxy")
                nc.vector.tensor_scalar(cxy[:], xy0[:], -1.0, float(W), Alu.max, Alu.min)
                aa = bpool.tile([128, U, NG], f32, tag="aa")
                nc.vector.tensor_scalar(aa[:], cxy[:, :, :, 0], float(W + 1), 0.0,
                                        Alu.add, Alu.add)
                an = bpool.tile([128, NG, U], f32, tag="an")
                anv = an[:].rearrange("p g u -> p u g")
                nc.vector.scalar_tensor_tensor(anv, cxy[:, :, :, 1], float(W), aa[:],
                                               Alu.mult, Alu.add)
                ang = bpool.tile([128, GRP], f32, tag="ang")
                nc.vector.tensor_tensor(ang[:], an[:].rearrange("p g u -> p (g u)"),
                                        hoffb[:], Alu.add)

                if DBG and ft == 0:
                    nc.sync.dma_start(
                        dbg_oa[:].rearrange("p u c -> p (u c)").rearrange(
                            "p (u c) -> p u c", c=48)[:, :, 0:32],
                        off_t[:].rearrange("p u g c -> p u (g c)"))
                    nc.sync.dma_start(
                        dbg_oa[:].rearrange("p u c -> p u c")[:, :, 32:48], att[:])
                    nc.sync.dma_start(dbg_ang[:], ang[:])
                    nc.sync.dma_start(dbg_w4[:], w4[:])
                # anchor fold to dma_gather index layout + replicate
                pf = ps_wide.tile([16, 8, GRP], f32, tag="pswide")
                for a in range(8):
                    nc.tensor.matmul(pf[:, a, :], sels[:, a * 16:(a + 1) * 16],
                                     ang[:], start=True, stop=True)
                ifold = qpool.tile([16, GRP, 8], f32, tag="ifold")
                nc.vector.tensor_copy(ifold[:].rearrange("p g a -> p a g"), pf[:])
                pi = ps_wide.tile([128, GRP * 8], f32, tag="pswide")
                iflat = ifold[:].rearrange("p g a -> p (g a)")
                for off in range(0, GRP * 8, 256):
                    nc.tensor.matmul(pi[:, off:off + 256], rep[:],
                                     iflat[:, off:off + 256], start=True, stop=True)
                i16t = qpool.tile([128, GRP * 8], i16, tag="i16t")
                nc.vector.tensor_copy(i16t[:], pi[:])

                # gathers (one per head pair)
                g_t = gpool.tile([128, GRP, 128], f32, tag="gt_")
                if cfg.get("no_gather"):
                    nc.vector.memset(g_t[:], 0.0)
                else:
                    CH = 1024                  # max num_idxs per dma_gather
                    nch = NIH // CH
                    for p in range(2):
                        src = bass.AP(tmap[p][:].tensor, 0,
                                      [[64, 2 * TR - 1], [1, 128]])
                        for k in range(nch):
                            gbase = p * (GRP // 2) + k * (CH // 128)
                            cbase = p * (GRP * 4) + k * (CH // 16)
                            nc.gpsimd.dma_gather(
                                g_t[:, gbase:gbase + CH // 128, :], src,
                                i16t[:, cbase:cbase + CH // 16],
                                CH, CH, 128, elem_step=64)

                if DBG and ft == 0:
                    nc.sync.dma_start(dbg_i16[:], i16t[:])
                    nc.sync.dma_start(dbg_g[:], g_t[:])
                # weighted combine: M2[p, grp, ch, c] = G * W4 (bcast ch),
                # one op per corner, grp-range split between DVE and GPSIMD
                m_t = mpool.tile([128, GRP, 32, 4], f32, tag="mt", bufs=1)
                for c in range(4):
                    dst = m_t[:, :, :, c]
                    src_g = g_t[:, :, c * 32:(c + 1) * 32]
                    src_w = w4[:].rearrange("p g u c -> p (g u) c")[
                        :, :, c].unsqueeze(2).broadcast_to([128, GRP, 32])
                    nc.vector.tensor_tensor(dst, src_g, src_w, Alu.mult)
                # reduce corners (innermost) then points (pairwise adds)
                s1 = mpool.tile([128, GRP, 32], f32, tag="s1")
                nc.vector.tensor_reduce(
                    s1[:].rearrange("p g ch -> p (g ch)"),
                    m_t[:].rearrange("p g ch c -> p (g ch) c"), AX.X, Alu.add)
                s1v = s1[:].rearrange("p (h t u) ch -> p t h (u ch)", t=4, u=U)
                pa = mpool.tile([128, 4, U * 32], f32, tag="pa")
                pb = mpool.tile([128, 4, U * 32], f32, tag="pb")
                nc.vector.tensor_tensor(pa[:], s1v[:, 0], s1v[:, 1], Alu.add)
                nc.vector.tensor_tensor(pb[:], s1v[:, 2], s1v[:, 3], Alu.add)
                smp = mpool.tile([128, U, 4, 32], f32, tag="smp")
                pav = pa[:].rearrange("p h (u ch) -> p h u ch", ch=32)
                pbv = pb[:].rearrange("p h (u ch) -> p h u ch", ch=32)
                smpv = smp[:].rearrange("p u h ch -> p h u ch")
                for u in range(U):
                    nc.vector.tensor_tensor(smpv[:, :, u, :], pav[:, :, u, :],
                                            pbv[:, :, u, :], Alu.add)

                if DBG and ft == 0:
                    nc.sync.dma_start(dbg_smp[:], smp[:])
                # output projection per u-slice
                ouf = qpool.tile([128, U, D], f32, tag="ouf")
                for u in range(U):
                    pt_ = ps_sm.tile([128, 128], f32, tag="pssm")
                    nc.tensor.transpose(pt_[:], smp[:, u], idn[:])
                    st_ = qpool.tile([128, 128], f32, tag="st_")
                    nc.scalar.copy(st_[:], pt_[:])
                    po = ps_v.tile([128, D], f32, tag="psv")
                    nc.tensor.matmul(po[:], st_[:], wo_sb[:], start=True, stop=True)
                    nc.vector.tensor_copy(ouf[:, u, :], po[:])
                nc.sync.dma_start(out_v[ft], ouf[:])

    nc.compile()
    return nc


# ---------------------------------------------------------------- host side

def _prep_consts(cfg):
    U, GRP, TR = cfg["U"], cfg["GRP"], cfg["TR"]
    sels = np.zeros((128, 128), np.float32)
    for a in range(8):
        for p in range(16):
            sels[p + 16 * a, a * 16 + p] = 1.0
    rep = np.zeros((16, 128), np.float32)
    for m in range(128):
        rep[m % 16, m] = 1.0
    idn = np.eye(128, dtype=np.float32)
    hoffb = np.zeros((128, GRP), np.float32)
    for g in range(16):
        h = g // 4
        hoffb[:, g * U:(g + 1) * U] = (h % 2) * TR
    return dict(sels=sels, rep=rep, idn=idn, hoffb=hoffb)


def make_core_inputs(cfg, inputs, b, hg):
    """Build the input map for core (b, hg) from the full problem inputs."""
    QP, SP, D = cfg["QP"], cfg["SP"], cfg["D"]
    Q, S = cfg["Q"], cfg["S"]

    def pad_rows(x, n):
        if x.shape[0] == n:
            return np.ascontiguousarray(x, dtype=np.float32)
        out = np.zeros((n,) + x.shape[1:], np.float32)
        out[:x.shape[0]] = x
        return out

    W_off, b_off = inputs["W_off"], inputs["b_off"]
    W_attn, b_attn = inputs["W_attn"], inputs["b_attn"]
    W_val, W_out = inputs["W_val"], inputs["W_out"]

    wofa = np.concatenate([W_off[hg * 32:(hg + 1) * 32],
                           W_attn[hg * 16:(hg + 1) * 16]], axis=0).T
    bofa = np.concatenate([b_off[hg * 32:(hg + 1) * 32],
                           b_attn[hg * 16:(hg + 1) * 16]])[None, :]
    wv = W_val[hg * 128:(hg + 1) * 128, :].T
    wo = W_out[:, hg * 128:(hg + 1) * 128].T

    m = dict(
        hidden=pad_rows(np.asarray(inputs["hidden_states"][b]), QP),
        encoder=pad_rows(np.asarray(inputs["encoder_hidden_states"][b]), SP),
        ref=pad_rows(np.asarray(inputs["reference_points"][b, :, 0, :]), QP),
        wofa=np.ascontiguousarray(wofa, np.float32),
        bofa=np.ascontiguousarray(bofa, np.float32),
        wv=np.ascontiguousarray(wv, np.float32),
        wo=np.ascontiguousarray(wo, np.float32),
    )
    m.update(_prep_consts(cfg))
    return m


_BUILT = {}


def _get_built(cfg_key=None):
    import sys
    sys.setrecursionlimit(100000)
    cfg = CFG_FULL
    key = "full"
    if key not in _BUILT:
        _BUILT[key] = build(cfg)
    return cfg, _BUILT[key]


def kernel(**inputs):
    from concourse.bass_utils import run_bass_kernel_spmd

    cfg, nc = _get_built()
    Q, D = cfg["Q"], cfg["D"]
    B = int(inputs["hidden_states"].shape[0])

    in_maps = []
    for core in range(8):
        b, hg = core // 2, core % 2
        in_maps.append(make_core_inputs(cfg, inputs, b, hg))

    res = run_bass_kernel_spmd(nc, in_maps, list(range(8))).results

    b_out = np.asarray(inputs["b_out"], np.float32)
    out = np.zeros((B, Q, D), np.float32)
    for b in range(B):
        out[b] = (np.asarray(res[2 * b]["outp"])[:Q]
                  + np.asarray(res[2 * b + 1]["outp"])[:Q] + b_out)
    return out
